# revision 39
# baseline (speedup 1.0000x reference)
"""Trainium2 Bass kernel for GQA (nn_GQA_28561532518475).

8 cores = 4 batches x 2 kv-head halves.  perm is folded into the weights on
the host (Wq cols -> slot order * scale, Wk/Wv rows by argsort(perm), Wp rows
by perm), so the device kernel is a plain GQA.  All matmuls run in bf16
(fp32 PSUM accumulation): halves LDWEIGHTS/SBUF/DMA traffic vs fp32 and runs
the PE at lower power (no HAM/thermal half-rate clamping).

Per core: qT and a partition-duplicated kT in [hd, P] layout, v natural with
a ones column (so the attention matmul also emits the softmax denominator).
The attention inner loop is ACT-bound (exp ~1.12us per 128x1024 key tile vs
~0.9us of PE work), runs a lag-2 software pipeline (scores(i) is emitted two
iterations ahead of AV(i-2)) so the PE never waits on exp and the ACT queue
stays fed across block boundaries.  All other PE work runs as (slot, fn)
fillers inside the loop:
  - the second half of v_proj and k0's key-half ride inside block 1,
    overlapping the second-half x DMA;
  - later q/k projections run as 6-matmul half-groups spread over blocks,
    each emitted at least one block before its first consumer (k halves
    are KEYS - needed from iteration 8 of the head's FIRST block);
  - the output projection is transposed (yT = Wp^T @ outT) in 3-matmul
    groups; the 12 needing only jq=0 attention output run inside kv head
    2's late blocks, the other 12 in the epilogue (host transposes back).
Softmax: the AV accumulator (PSUM) is copied to SBUF immediately so its
single PSUM buffer frees fast; each block's normalize is deferred into the
next block: ones-matmul broadcast of the denominator row over 64 psum
partitions, reciprocal_approx_fast on DVE (full-rate reciprocal is 6.5us
and would serialize the DVE queue), multiply on the idle GpSimd engine.
Dummy warm-up matmuls at t=0 and before the epilogue hold the PE HAM
clock-gate at 2.4GHz through the DMA fill and the output-projection tail.
Host sums the two partial outputs per batch + bias.
"""

import numpy as np

B, P, C = 4, 2048, 768
H, HK, HD, GS = 12, 6, 64, 2
SCALE = HD ** -0.5
NKV = 3          # kv heads per core
NH = 6           # q heads per core
KT = C // 128    # 6 contraction tiles
PT = P // 128    # 16 row tiles
QB = 1024        # q-block width for attention
NQB = P // QB    # 2

_cached_nc = None


def _build_program():
    global _cached_nc
    if _cached_nc is not None:
        return _cached_nc

    import concourse.bass as bass
    import concourse.mybir as mybir
    import concourse.tile as tile
    from concourse import bacc

    fp32 = mybir.dt.float32
    bf16 = mybir.dt.bfloat16
    EXP = mybir.ActivationFunctionType.Exp

    nc = bacc.Bacc("TRN2", target_bir_lowering=False, debug=False)
    xT = nc.dram_tensor("xT", [C, P], bf16, kind="ExternalInput").ap()
    wq = nc.dram_tensor("wq", [C, 384], bf16, kind="ExternalInput").ap()
    wkd = nc.dram_tensor("wkd", [C, 384], bf16, kind="ExternalInput").ap()
    wv = nc.dram_tensor("wv", [C, 192], bf16, kind="ExternalInput").ap()
    wp = nc.dram_tensor("wp", [384, C], bf16, kind="ExternalInput").ap()
    fp32r = mybir.dt.float32r
    yT = nc.dram_tensor("yT", [C, P], bf16, kind="ExternalOutput").ap()

    with tile.TileContext(nc) as tc:
        from contextlib import ExitStack

        with ExitStack() as ctx:
            wpool = ctx.enter_context(tc.tile_pool(name="weights", bufs=1))
            qkvp = ctx.enter_context(tc.tile_pool(name="qkv", bufs=1))
            outp = ctx.enter_context(tc.tile_pool(name="outT", bufs=1))
            epool = ctx.enter_context(tc.tile_pool(name="E", bufs=3))
            nrmp = ctx.enter_context(tc.tile_pool(name="norm", bufs=2))
            obsp = ctx.enter_context(tc.tile_pool(name="obs", bufs=2))
            ysbp = ctx.enter_context(tc.tile_pool(name="ysb", bufs=4))

            # x chunks: jq=0 half first so v_proj/qk(0) start early; wv
            # before x (v_proj needs it first), wp last (epilogue only)
            # x arrives token-major (8 chunks of 256 tokens x all 6 k-tiles)
            # round-robin over the three DMA-capable engines so v_proj can
            # start after the first ~0.4MB
            xt = wpool.tile([128, KT, P], bf16)
            wv_sb = wpool.tile([128, KT, 192], bf16)
            nc.sync.dma_start(wv_sb[:], wv.rearrange("(t p) n -> p t n", p=128))
            dma_engs = (nc.sync, nc.scalar, nc.gpsimd)
            for kc in range(KT):
                dma_engs[kc % 3].dma_start(
                    xt[:, kc, 0:QB],
                    xT[kc * 128 : (kc + 1) * 128, 0:QB],
                )
            wkd_sb = wpool.tile([128, KT, 384], bf16)
            nc.scalar.dma_start(wkd_sb[:], wkd.rearrange("(t p) n -> p t n", p=128))
            wq_sb = wpool.tile([128, KT, 384], bf16)
            nc.gpsimd.dma_start(wq_sb[:], wq.rearrange("(t p) n -> p t n", p=128))
            for kc in range(KT):
                dma_engs[kc % 3].dma_start(
                    xt[:, kc, QB : 2 * QB],
                    xT[kc * 128 : (kc + 1) * 128, QB : 2 * QB],
                )
            wp_sb = wpool.tile([128, 3, C], bf16)
            nc.sync.dma_start(wp_sb[:], wp.rearrange("(t p) n -> p t n", p=128))

            # HAM warm-up: ~4us of dummy matmuls at t=0 (overlapping the
            # DMA fill) so the PE clock-gate is at 2.4GHz when real work
            # arrives; result is never read
            with tc.tile_pool(name="warm", bufs=1, space="PSUM") as wmp:
                warm_w = wpool.tile([128, 512], bf16, name="warm_w", tag="warm_w")
                nc.vector.memset(warm_w[:], 1.0)
                wps = wmp.tile([128, 512], fp32, tag="w")
                for _ in range(16):
                    nc.tensor.matmul(
                        wps[0:64, :],
                        warm_w[:, 0:64],
                        warm_w[:, :],
                        start=True,
                        stop=True,
                    )

            qts = [qkvp.tile([128, P], bf16, name=f"qt{m}", tag=f"qt{m}") for m in range(NKV)]
            kts = [qkvp.tile([128, P], bf16, name=f"kt{m}", tag=f"kt{m}") for m in range(NKV)]
            vexts = [
                qkvp.tile([128, PT, HD + 1], bf16, name=f"vx{m}", tag=f"vx{m}")
                for m in range(NKV)
            ]
            for m in range(NKV):
                nc.vector.memset(vexts[m][:, :, HD], 1.0)
            # fp32r ones row used to broadcast the softmax denominator
            # across 64 PSUM partitions via a Kc=1 matmul
            ones_r = qkvp.tile([128, 64], fp32r, name="ones_r", tag="ones_r")
            nc.vector.memset(ones_r[:].bitcast(fp32), 1.0)
            outTs = [outp.tile([128, P], bf16, name=f"oT{m}", tag=f"oT{m}") for m in range(NKV)]

            with tc.tile_pool(name="mm_ps", bufs=2, space="PSUM") as sps, tc.tile_pool(
                name="o_ps", bufs=1, space="PSUM"
            ) as ops, tc.tile_pool(name="p_ps", bufs=1, space="PSUM") as pps:

                proj_ps = {}

                def qk_half(kv, which, nb, half):
                    """6 matmuls: one 512-wide half of the q or k projection
                    for head kv, query block nb.  half==1 also writes out the
                    finished [128, 1024] tile."""
                    w_sb = wq_sb if which == 0 else wkd_sb
                    dest = qts[kv] if which == 0 else kts[kv]
                    key = (kv, which, nb)
                    if half == 0:
                        proj_ps[key] = pps.tile([128, QB], fp32, name="pj", tag="p")
                        ps = proj_ps[key]
                    else:
                        ps = proj_ps.pop(key)
                    for kc in range(KT):
                        nc.tensor.matmul(
                            ps[:, half * 512 : (half + 1) * 512],
                            w_sb[:, kc, kv * 128 : (kv + 1) * 128],
                            xt[:, kc, nb * QB + half * 512 : nb * QB + (half + 1) * 512],
                            start=(kc == 0),
                            stop=(kc == KT - 1),
                        )
                    if half == 1:
                        nc.vector.tensor_copy(dest[:, nb * QB : (nb + 1) * QB], ps[:])

                def y_group(cc, tb, pool, wide):
                    """Transposed output projection: yT[cc*128:+128,
                    tb*512:+512] = sum_kf wp_chunk^T @ outT chunk."""
                    if wide:
                        ps = pool.tile([128, QB], fp32, name="pj", tag="p")[:, 0:512]
                    else:
                        ps = pool.tile([128, 512], fp32, tag="y")
                    for kf in range(3):
                        nc.tensor.matmul(
                            ps[:],
                            wp_sb[:, kf, cc * 128 : (cc + 1) * 128],
                            outTs[kf][:, tb * 512 : (tb + 1) * 512],
                            start=(kf == 0),
                            stop=(kf == 2),
                        )
                    ysb = ysbp.tile([128, 512], bf16, tag="y")
                    nc.vector.tensor_copy(ysb[:], ps[:])
                    # filler groups run while ACT is busy: keep their DMAs off
                    # the Activation queue
                    eng = nc.sync if (wide or (cc + tb) % 2 == 0) else nc.scalar
                    eng.dma_start(
                        yT[cc * 128 : (cc + 1) * 128, tb * 512 : (tb + 1) * 512],
                        ysb[:],
                    )

                def v_pair(i, j):
                    """Two v_proj stripes sharing one projection-pool PSUM
                    slot (block 1 filler; the slot is otherwise idle there)."""
                    ps = pps.tile([128, QB], fp32, name="pj", tag="p")
                    for kc in range(KT):
                        for col, st in ((0, i), (512, j)):
                            nc.tensor.matmul(
                                ps[:, col : col + 192],
                                xt[:, kc, st * 128 : (st + 1) * 128],
                                wv_sb[:, kc, :],
                                start=(kc == 0),
                                stop=(kc == KT - 1),
                            )
                    for col, st in ((0, i), (512, j)):
                        psv = ps[:, col : col + 192].rearrange("p (h d) -> p h d", h=NKV)
                        for m in range(NKV):
                            nc.vector.tensor_copy(vexts[m][:, st, 0:HD], psv[:, m, :])

                def v_proj(i):
                    ps = sps.tile([128, 192], fp32, name="vp", tag="s")
                    for kc in range(KT):
                        nc.tensor.matmul(
                            ps[:],
                            xt[:, kc, i * 128 : (i + 1) * 128],
                            wv_sb[:, kc, :],
                            start=(kc == 0),
                            stop=(kc == KT - 1),
                        )
                    psv = ps.rearrange("p (h d) -> p h d", h=NKV)
                    for m in range(NKV):
                        nc.vector.tensor_copy(vexts[m][:, i, 0:HD], psv[:, m, :])

                def normalize(kv, g, q0, obs):
                    """Deferred softmax-denominator apply for a finished
                    block: broadcast the denominator row over 64 partitions
                    with a Kc=1 ones matmul, reciprocal on DVE, multiply into
                    outT.  Emitted inside the NEXT block (after its first
                    scores matmul) so the PE never stalls on the obs copy."""
                    rbp = pps.tile([128, QB], fp32, name="pj", tag="p")
                    for nb in range(QB // 512):
                        nc.tensor.matmul(
                            rbp[0:HD, nb * 512 : (nb + 1) * 512],
                            ones_r[HD : HD + 1, :],
                            obs[HD : HD + 1, nb * 512 : (nb + 1) * 512],
                            start=True,
                            stop=True,
                        )
                    rb = nrmp.tile([HD, QB], fp32, tag="rb")
                    nc.vector.reciprocal_approx_fast(rb[:], rbp[0:HD, :])
                    if g == 0:
                        nc.gpsimd.tensor_mul(
                            outTs[kv][0:HD, q0 : q0 + QB], obs[0:HD, :], rb[:]
                        )
                    else:
                        sc2 = nrmp.tile([HD, QB], bf16, tag="sc2")
                        nc.gpsimd.tensor_mul(sc2[:], obs[0:HD, :], rb[:])
                        nc.sync.dma_start(
                            outTs[kv][HD:128, q0 : q0 + QB], sc2[:]
                        )

                def attention_block(kv, g, jq, fillers, prev_norm):
                    """16 key tiles of scores->exp->AV in a lag-1 software
                    pipeline: scores(i+1) is emitted before AV(i) so the PE
                    never waits on exp(i) and the ACT queue stays fed across
                    the block boundary (AV(0) hides the previous block's
                    PSUM-accumulator release).  `fillers` is (slot, fn) PE
                    work run after AV(slot); `prev_norm` is the previous
                    block's deferred normalize.  Returns this block's
                    deferred normalize closure."""
                    gp = slice(g * 64, (g + 1) * 64)
                    q0 = jq * QB
                    ob = ops.tile([HD + 1, QB], fp32, tag="ob")
                    fq = {}
                    for i, fn in fillers:
                        fq.setdefault(i, []).append(fn)
                    es = {}

                    def scores(i):
                        s = sps.tile([128, QB], fp32, name="sc", tag="s")
                        for nb in range(QB // 512):
                            nc.tensor.matmul(
                                s[:, nb * 512 : (nb + 1) * 512],
                                kts[kv][gp, i * 128 : (i + 1) * 128],
                                qts[kv][gp, q0 + nb * 512 : q0 + (nb + 1) * 512],
                                start=True,
                                stop=True,
                            )
                        e = epool.tile([128, QB], bf16, tag="e")
                        nc.scalar.activation(e[:], s[:], EXP)
                        es[i] = e

                    def av(i):
                        e = es.pop(i)
                        for nb in range(QB // 512):
                            nc.tensor.matmul(
                                ob[:, nb * 512 : (nb + 1) * 512],
                                vexts[kv][:, i, :],
                                e[:, nb * 512 : (nb + 1) * 512],
                                start=(i == 0),
                                stop=(i == PT - 1),
                            )
                        for fn in fq.pop(i, ()):
                            fn()

                    scores(0)
                    scores(1)
                    if prev_norm is not None:
                        prev_norm()
                    for i in range(2, PT):
                        scores(i)
                        av(i - 2)
                    av(PT - 2)
                    av(PT - 1)
                    # free the single PSUM accumulator fast: one copy to SBUF
                    obs = obsp.tile([HD + 1, QB], fp32r, tag="obs")
                    nc.vector.tensor_copy(obs[:], ob[:])
                    return lambda: normalize(kv, g, q0, obs)

                def qk_v_half(which, nb, half, vstripes):
                    """qk_half with v_proj stripes interleaved per k-tile so
                    the short v matmuls (and their LDWEIGHTS) hide under the
                    512-wide qk streams."""
                    w_sb = wq_sb if which == 0 else wkd_sb
                    dest = qts[0] if which == 0 else kts[0]
                    key = (0, which, nb)
                    if half == 0:
                        proj_ps[key] = pps.tile([128, QB], fp32, name="pj", tag="p")
                        ps = proj_ps[key]
                    else:
                        ps = proj_ps.pop(key)
                    vps = [sps.tile([128, 192], fp32, name="vp", tag="s") for _ in vstripes]
                    for kc in range(KT):
                        nc.tensor.matmul(
                            ps[:, half * 512 : (half + 1) * 512],
                            w_sb[:, kc, 0:128],
                            xt[:, kc, nb * QB + half * 512 : nb * QB + (half + 1) * 512],
                            start=(kc == 0),
                            stop=(kc == KT - 1),
                        )
                        for i, vp in zip(vstripes, vps):
                            nc.tensor.matmul(
                                vp[:],
                                xt[:, kc, i * 128 : (i + 1) * 128],
                                wv_sb[:, kc, :],
                                start=(kc == 0),
                                stop=(kc == KT - 1),
                            )
                    if half == 1:
                        nc.scalar.activation(
                            dest[:, nb * QB : (nb + 1) * QB], ps[:],
                            mybir.ActivationFunctionType.Copy,
                        )
                    for i, vp in zip(vstripes, vps):
                        psv = vp.rearrange("p (h d) -> p h d", h=NKV)
                        for m in range(NKV):
                            nc.vector.tensor_copy(vexts[m][:, i, 0:HD], psv[:, m, :])

                # ---- prologue: v_proj woven under k0/q0 projections
                # (k columns are KEYS: both halves of k(kv) must be written
                # before kv's first attention block; q nb1 only before jq=1)
                v_proj(0)
                v_proj(1)
                qk_v_half(1, 0, 0, (2, 3))
                qk_v_half(1, 0, 1, ())
                qk_v_half(0, 0, 0, ())
                qk_v_half(0, 0, 1, ())

                # ---- filler schedule: each entry is the work injected into
                # one attention block (block order per kv: (g,jq) =
                # (0,0),(1,0),(0,1),(1,1); jq=0 blocks first so the jq=1
                # blocks of kv 2 can host jq=0 output-projection groups).
                def qk2(kv, which, nb):
                    return [
                        (3, lambda: qk_half(kv, which, nb, 0)),
                        (10, lambda: qk_half(kv, which, nb, 1)),
                    ]

                def yg6(tbs):
                    return [
                        (3 + 2 * j, (lambda cc=cc, tb=tb: y_group(cc, tb, pps, True)))
                        for j, (cc, tb) in enumerate(tbs)
                    ]

                FILL = [
                    # kv0 (0,0): remaining v_proj stripes (pps is free here)
                    # plus k0 nb1, placed late so it rides the second-half
                    # x DMA without blocking early iterations
                    [(s, (lambda a=a, b=b: v_pair(a, b)))
                     for s, (a, b) in zip(
                         (1, 2, 4, 6, 7, 8), ((4, 5), (6, 7), (8, 9),
                                              (10, 11), (12, 13), (14, 15)))]
                    + [(3, lambda: qk_half(0, 1, 1, 0)),
                       (5, lambda: qk_half(0, 1, 1, 1))],
                    # kv0 (1,0) .. (1,1): q0 nb1, then k1 (both) and q1 nb0
                    [(3, lambda: qk_half(0, 0, 1, 0)),
                     (7, lambda: qk_half(0, 0, 1, 1)),
                     (11, lambda: qk_half(1, 1, 0, 0))],
                    [(3, lambda: qk_half(1, 1, 0, 1)),
                     (7, lambda: qk_half(1, 1, 1, 0)),
                     (11, lambda: qk_half(1, 1, 1, 1))],
                    qk2(1, 0, 0),                    # kv0 (1,1): q1 nb0
                    qk2(1, 0, 1),                    # kv1 (0,0): q1 nb1
                    qk2(2, 1, 0),                    # kv1 (1,0): k2 nb0
                    qk2(2, 1, 1),                    # kv1 (0,1): k2 nb1
                    qk2(2, 0, 0),                    # kv1 (1,1): q2 nb0
                    qk2(2, 0, 1),                    # kv2 (0,0): q2 nb1
                    [],                              # kv2 (1,0)
                    yg6([(cc, tb) for cc in range(3) for tb in range(2)]),
                    yg6([(cc, tb) for cc in range(3, 6) for tb in range(2)]),
                ]
                # last kv head ends on a g=0 block so the final normalize
                # (critical tail before the jq=1 output projection) has no
                # partition-shift DMA
                bidx = 0
                pending_norm = None
                for kv in range(NKV):
                    blocks = ((0, 0), (1, 0), (0, 1), (1, 1)) if kv < 2 else (
                        (0, 0), (1, 0), (1, 1), (0, 1))
                    for g, jq in blocks:
                        pending_norm = attention_block(
                            kv, g, jq, FILL[bidx], pending_norm
                        )
                        bidx += 1
                pending_norm()

            # ---------------- remaining output projection (jq=1) ----------
            with tc.tile_pool(name="y_ps", bufs=4, space="PSUM") as yps:
                # keep the PE clock-gate warm while the final normalize
                # chain (DVE/gpsimd) runs; results never read
                wps2 = yps.tile([128, 512], fp32, tag="y")
                for _ in range(14):
                    nc.tensor.matmul(
                        wps2[0:64, :],
                        warm_w[:, 0:64],
                        warm_w[:, :],
                        start=True,
                        stop=True,
                    )
                for cc in range(6):
                    for tb in range(2, 4):
                        y_group(cc, tb, yps, False)

    nc.compile()
    _cached_nc = nc
    return nc


def _make_in_maps(x, Wq, Wk, Wv, Wp, perm):
    import ml_dtypes

    bf16 = ml_dtypes.bfloat16
    inv = np.argsort(perm)
    Wq_f = np.ascontiguousarray(
        Wq.reshape(C, H, HD)[:, perm, :].reshape(C, C) * SCALE
    ).astype(bf16)
    Wk_f = Wk.reshape(H, HD, HK * HD)[inv].reshape(C, HK * HD).astype(bf16)
    Wv_f = Wv.reshape(H, HD, HK * HD)[inv].reshape(C, HK * HD).astype(bf16)
    Wp_f = Wp.reshape(H, HD, C)[perm].reshape(C, C).astype(bf16)

    in_maps = []
    for core in range(8):
        b, half = core // 2, core % 2
        wk_half = Wk_f[:, half * 192 : (half + 1) * 192].reshape(C, NKV, 1, HD)
        wkd = np.ascontiguousarray(
            np.broadcast_to(wk_half, (C, NKV, 2, HD)).reshape(C, 384)
        )
        in_maps.append(
            {
                "xT": np.ascontiguousarray(x[b].T).astype(bf16),
                "wq": np.ascontiguousarray(Wq_f[:, half * 384 : (half + 1) * 384]),
                "wkd": wkd,
                "wv": np.ascontiguousarray(Wv_f[:, half * 192 : (half + 1) * 192]),
                "wp": np.ascontiguousarray(Wp_f[half * 384 : (half + 1) * 384, :]),
            }
        )
    return in_maps


def kernel(x, Wq, Wk, Wv, Wp, bp, bass_run_kwargs=None, **_unused):
    perm = _unused.pop("perm")
    from concourse.bass_utils import run_bass_kernel_spmd

    x = np.asarray(x, np.float32)
    nc = _build_program()
    in_maps = _make_in_maps(
        x,
        np.asarray(Wq, np.float32),
        np.asarray(Wk, np.float32),
        np.asarray(Wv, np.float32),
        np.asarray(Wp, np.float32),
        np.asarray(perm),
    )
    res = run_bass_kernel_spmd(
        nc, in_maps, core_ids=list(range(8)), **(bass_run_kwargs or {})
    )
    bp = np.asarray(bp, np.float32)
    yout = np.empty((B, P, C), np.float32)
    for b in range(B):
        yout[b] = (
            res.results[2 * b]["yT"].astype(np.float32).T
            + res.results[2 * b + 1]["yT"].astype(np.float32).T
            + bp
        )
    if bass_run_kwargs:
        kernel.last_results = res
    return yout
